# revision 1
# baseline (speedup 1.0000x reference)
"""Trainium2 Bass kernel for an Elman RNN language model (raw bass, SPMD x8).

Model (per reference):
    X = lookup[input_batch]                      # [S, B, E]
    h_t = tanh(x_t @ Wx + h_{t-1} @ Wh)          # [B, H]
    out_t = log_softmax(h_t @ Wo, axis=-1)       # [B, V]
    output: [S, B, V] float32,  S=128 B=64 V=32000 E=32 H=16

Sharding: data-parallel over batch, 8 batch rows per core. Each core
produces its [S, 8, V] slice (131 MB) — memory-bound on output writes.

Per-core program (raw bass, explicit single-wait semaphores):
  * embedding rows via indirect DMA gather, PE-transposed to xt [E, R]
  * sequential recurrence; the Wx/Wh stationaries are host-replicated
    across four 32-row PE strips so the hidden state lands replicated
    on partitions {0,32,64,96}+0..15; tanh synthesized from exp
    (1 - 2/(exp(2z)+1)) so all ACT ops share one table set
  * Wo is host-packed into 4 vocab quarters on PE strips ([128, 8000]
    f32, full-bandwidth DMA) and rounded to f32r by one DVE copy
  * per 128-row block rb: pass A = logits chunks (fp32r strip matmul at
    1 cycle/row via tile_position) + ACT exp(accum_out) -> sums -> ln;
    pass B = recompute logits through a 3-bank PSUM rotation and fuse
    the -logZ subtract into the DVE PSUM->SBUF copy; 4 MB staged
    output DMAs on two alternating staging slots
  * phases are software-pipelined by row block: pass B of rb-1, pass A
    of rb, and the recurrence of rb+1 run concurrently
"""

import numpy as np

import concourse.bass as bass
import concourse.mybir as mybir
from concourse.bass_utils import run_bass_kernel_spmd

F32 = mybir.dt.float32
F32R = mybir.dt.float32r
I32 = mybir.dt.int32

S, B, V, E, H = 128, 64, 32000, 32, 16
NCORES = 8
BL = B // NCORES          # 8 batch rows per core
R = S * BL                # 1024 rows per core, t-major (row = t*8 + j)
RBP = 128                 # rows per row block (16 timesteps)
NRB = R // RBP            # 8
CH = 500                  # vocab chunk, one matmul
NCH = V // CH             # 64 chunks per row block
PPB = NCH // 2            # 32 exp pairs per row block
QV = V // 4               # 8000 vocab cols per PE strip quarter
STG = 8000                # staging cols per output DMA (4 MB per DMA)
NSTG = V // STG           # 4 staged DMAs per row block
CPS = STG // CH           # 16 chunks per staging group
GAT_INC = 16
OUT_INC = 16

Exp = mybir.ActivationFunctionType.Exp
Ln = mybir.ActivationFunctionType.Ln
Identity = mybir.ActivationFunctionType.Identity
Add = mybir.AluOpType.add
Sub = mybir.AluOpType.subtract
Mult = mybir.AluOpType.mult
AxX = mybir.AxisListType.X


def build_module():
    nc = bass.Bass()

    idx_d = nc.declare_dram_parameter("idx", [RBP, NRB], I32, isOutput=False)
    lookup_d = nc.declare_dram_parameter("lookup", [V, E], F32, isOutput=False)
    wx_d = nc.declare_dram_parameter("wxr", [E, RBP], F32, isOutput=False)
    wh_d = nc.declare_dram_parameter("whr", [H, RBP], F32, isOutput=False)
    wh2_d = nc.declare_dram_parameter("whr2", [H + 1, RBP], F32, isOutput=False)
    wo_d = nc.declare_dram_parameter("woq", [RBP, QV], F32, isOutput=False)
    h0t_d = nc.declare_dram_parameter("h0t", [H, BL], F32, isOutput=False)
    ident_d = nc.declare_dram_parameter("ident", [RBP, RBP], F32, isOutput=False)
    out_d = nc.declare_dram_parameter("out", [R, V], F32, isOutput=True)

    # ---- SBUF ----
    wx_sb = nc.alloc_sbuf_tensor("wx_sb", [E, RBP], F32)
    wh_sb = nc.alloc_sbuf_tensor("wh_sb", [H, RBP], F32)
    wh2_sb = nc.alloc_sbuf_tensor("wh2_sb", [H + 1, RBP], F32)
    h0t_sb = nc.alloc_sbuf_tensor("h0t_sb", [H, BL], F32)
    wo_f = nc.alloc_sbuf_tensor("wo_f", [RBP, QV], F32)
    wo_r = nc.alloc_sbuf_tensor("wo_r", [RBP, QV], F32R)
    ident = nc.alloc_sbuf_tensor("ident_sb", [RBP, RBP], F32)
    idx_sb = nc.alloc_sbuf_tensor("idx_sb", [RBP, NRB], I32)
    xg = nc.alloc_sbuf_tensor("xg", [RBP, NRB * E], F32)
    xt = nc.alloc_sbuf_tensor("xt", [E, R], F32)
    hall = nc.alloc_sbuf_tensor("hall", [RBP, R], F32)
    hall_r = nc.alloc_sbuf_tensor("hall_r", [RBP, R], F32R)
    e_sb = nc.alloc_sbuf_tensor("e_sb", [RBP, 2 * BL], F32)
    u_sb = nc.alloc_sbuf_tensor("u_sb", [RBP, BL], F32)
    r_sb = nc.alloc_sbuf_tensor("r_sb", [RBP, BL], F32)
    esums = nc.alloc_sbuf_tensor("esums", [RBP, 2 * PPB], F32)
    rsum = nc.alloc_sbuf_tensor("rsum", [RBP, NRB], F32)
    logz = nc.alloc_sbuf_tensor("logz", [RBP, NRB], F32)
    nlogz = nc.alloc_sbuf_tensor("nlogz", [RBP, NRB], F32)
    expdump = nc.alloc_sbuf_tensor("expdump", [RBP, 1024], F32)
    stg = nc.alloc_sbuf_tensor("stg", [RBP, 2 * STG], F32)

    # ---- PSUM (8 banks) ----
    pt = nc.alloc_psum_tensor("pt", [RBP, RBP], F32)                    # 1 bank
    pa = [nc.alloc_psum_tensor(f"pa{i}", [RBP, 1024], F32) for i in range(2)]  # 4
    pb = [nc.alloc_psum_tensor(f"pb{i}", [RBP, CH], F32) for i in range(3)]  # 3

    in_hw = nc.alloc_semaphore("in_hw")    # 4 SP input DMAs -> 64
    in_idx = nc.alloc_semaphore("in_idx")  # idx DMA
    in_wo = nc.alloc_semaphore("in_wo")    # wo DMA
    gats = [nc.alloc_semaphore(f"gat{i}") for i in range(NRB)]
    pe_xt = nc.alloc_semaphore("pe_xt")    # +1 per transpose
    dve_xt = nc.alloc_semaphore("dve_xt")  # +1 per xt copy
    dve_wo = nc.alloc_semaphore("dve_wo")  # +1 after wo f32r cast
    pe_rec = nc.alloc_semaphore("pe_rec")  # +1 per recurrence mm pair
    act_rec = nc.alloc_semaphore("act_rec")  # +1 per recurrence exp
    dve_h = nc.alloc_semaphore("dve_h")    # +1 per recurrence h write
    dve_hr = nc.alloc_semaphore("dve_hr")  # +1 per hall_r rowblock cast
    pe_paA = nc.alloc_semaphore("pe_paA")  # +1 per pass A matmul
    act_eA = nc.alloc_semaphore("act_eA")  # +1 per pass A exp PAIR
    dve_red = nc.alloc_semaphore("dve_red")  # +1 per esums reduce
    act_ln = nc.alloc_semaphore("act_ln")  # +1 per ln
    dve_nl = nc.alloc_semaphore("dve_nl")  # +1 per negate
    pe_pb = nc.alloc_semaphore("pe_pb")    # +1 per pass B matmul
    dve_cb = nc.alloc_semaphore("dve_cb")  # +1 per DVE pass B copy
    act_cb = nc.alloc_semaphore("act_cb")  # +1 per ACT pass B copy
    out_s = [nc.alloc_semaphore(f"out_s{i}") for i in range(2)]

    NG = NRB * NSTG          # 16 output DMAs / staging groups
    DPG = CPS - CPS // 8     # 28 DVE copies per group
    APG = CPS // 8           # 4 ACT copies per group

    def wo_sl(c):
        """(tile_position, rhs AP) for vocab chunk c (cols c*500..+500)."""
        q, cc = divmod(c, NCH // 4)
        return 32 * q, wo_r[32 * q:32 * q + H + 1, cc * CH:(cc + 1) * CH]

    def pa_view(t):
        return t[:].rearrange("p (b c) -> p b c", b=2)[:, :, 0:CH]

    with nc.Block() as block:
        @block.sync
        def _(sync):
            sync.dma_start(idx_sb[:], idx_d[:]).then_inc(in_idx, 16)
            sync.dma_start(wx_sb[:], wx_d[:]).then_inc(in_hw, 16)
            sync.dma_start(wh_sb[:], wh_d[:]).then_inc(in_hw, 16)
            sync.dma_start(wh2_sb[:], wh2_d[:]).then_inc(in_hw, 16)
            sync.dma_start(h0t_sb[:], h0t_d[:]).then_inc(in_hw, 16)
            sync.dma_start(ident[:], ident_d[:]).then_inc(in_hw, 16)
            sync.dma_start(wo_f[:], wo_d[:]).then_inc(in_wo, 16)
            for g in range(NG):
                rb, gg = divmod(g, NSTG)
                sync.wait_ge(dve_cb, CPS * (g + 1))
                sync.dma_start(
                    out_d[rb * RBP:(rb + 1) * RBP, gg * STG:(gg + 1) * STG],
                    stg[:, (g % 2) * STG:(g % 2 + 1) * STG],
                ).then_inc(out_s[g % 2], 16)
            sync.wait_ge(out_s[0], OUT_INC * (NG // 2))
            sync.wait_ge(out_s[1], OUT_INC * (NG // 2))

        @block.gpsimd
        def _(gpsimd):
            gpsimd.wait_ge(in_idx, 16)
            for rb in range(NRB):
                gpsimd.indirect_dma_start(
                    out=xg[:, rb * E:(rb + 1) * E],
                    out_offset=None,
                    in_=lookup_d[:],
                    in_offset=bass.IndirectOffsetOnAxis(
                        ap=idx_sb[:, rb:rb + 1], axis=0),
                ).then_inc(gats[rb], 16)

        @block.tensor
        def _(tensor):
            def rec_step(t):
                if t >= 1:
                    tensor.wait_ge(act_rec, t)   # pt bank freed by exp t-1
                pr = pt[:, 0:BL]
                nc.tensor.matmul(
                    pr, lhsT=wx_sb[:], rhs=xt[:, t * BL:(t + 1) * BL],
                    start=True, stop=False,
                )
                if t >= 1:
                    tensor.wait_ge(dve_h, t)     # r_{t-1} ready
                if t == 0:
                    nc.tensor.matmul(
                        pr, lhsT=wh_sb[:], rhs=h0t_sb[:],
                        start=False, stop=True,
                    ).then_inc(pe_rec, 1)
                else:
                    nc.tensor.matmul(
                        pr, lhsT=wh2_sb[:],
                        rhs=hall[0:H + 1, (t - 1) * BL:t * BL],
                        start=False, stop=True,
                    ).then_inc(pe_rec, 1)

            def passA_pair(rb, j):
                p = rb * PPB + j
                if j == 0:
                    tensor.wait_ge(dve_hr, rb + 1)
                    if rb == 0:
                        tensor.wait_ge(dve_wo, 1)
                if p >= 2:
                    tensor.wait_ge(act_eA, p - 1)  # pa[p%2] freed
                for half in range(2):
                    c = 2 * j + half
                    bp, rhs = wo_sl(c)
                    nc.tensor.matmul(
                        pa[p % 2][:, half * 512:half * 512 + CH],
                        lhsT=hall_r[bp:bp + H + 1, rb * RBP:(rb + 1) * RBP],
                        rhs=rhs, start=True, stop=True,
                        tile_position=(bp, 0),
                    ).then_inc(pe_paA, 1)

            nb = [0]          # global B-chunk counter

            def passB_chunk(rb, c):
                n = nb[0]
                nb[0] += 1
                if n >= 3:
                    tensor.wait_ge(dve_cb, n - 2)   # pb[n%3] freed by copy n-3
                bp, rhs = wo_sl(c)
                nc.tensor.matmul(
                    pb[n % 3][:],
                    lhsT=hall_r[bp:bp + H + 1, rb * RBP:(rb + 1) * RBP],
                    rhs=rhs, start=True, stop=True,
                    tile_position=(bp, 0),
                ).then_inc(pe_pb, 1)

            tensor.wait_ge(in_hw, 80)
            for k in range(NRB):
                if k >= 1:
                    tensor.wait_ge(dve_xt, k)    # pt freed by copy k-1
                tensor.wait_ge(gats[k], GAT_INC)
                nc.tensor.transpose(
                    out=pt[0:E, :], in_=xg[:, k * E:(k + 1) * E],
                    identity=ident[:],
                ).then_inc(pe_xt, 1)
            tensor.wait_ge(dve_xt, NRB)
            for t in range(16):
                rec_step(t)
            for slot in range(NRB):
                for i in range(PPB):
                    if slot >= 1:
                        passB_chunk(slot - 1, 2 * i)
                        passB_chunk(slot - 1, 2 * i + 1)
                    passA_pair(slot, i)
                    if slot + 1 < NRB and i % 2 == 0:
                        rec_step(16 * (slot + 1) + i // 2)
            for i in range(PPB):
                passB_chunk(NRB - 1, 2 * i)
                passB_chunk(NRB - 1, 2 * i + 1)

        @block.scalar
        def _(scalar):
            def rec_exp(t):
                if t >= 2:
                    scalar.wait_ge(dve_h, t - 1)  # e_sb slot freed
                scalar.wait_ge(pe_rec, t + 1)
                nc.scalar.activation(
                    e_sb[:, (t % 2) * BL:(t % 2 + 1) * BL],
                    pt[:, 0:BL], Exp, scale=2.0,
                ).then_inc(act_rec, 1)

            def expA_pair(rb, j):
                p = rb * PPB + j
                if j == 0 and rb >= 2:
                    scalar.wait_ge(dve_red, rb - 1)  # esums slot freed
                scalar.wait_ge(pe_paA, 2 * p + 2)
                if p >= 1:
                    nc.scalar.drain()                # expdump WAW
                nc.scalar.activation(
                    pa_view(expdump),
                    pa_view(pa[p % 2]), Exp,
                    accum_out=esums[:, (rb % 2) * PPB + j:(rb % 2) * PPB + j + 1],
                ).then_inc(act_eA, 1)

            def ln_rb(rb):
                scalar.wait_ge(dve_red, rb + 1)
                nc.scalar.activation(
                    logz[:, rb:rb + 1], rsum[:, rb:rb + 1], Ln,
                ).then_inc(act_ln, 1)

            for t in range(16):
                rec_exp(t)
            for slot in range(NRB):
                for i in range(PPB):
                    expA_pair(slot, i)
                    if slot + 1 < NRB and i % 2 == 0:
                        rec_exp(16 * (slot + 1) + i // 2)
                ln_rb(slot)

        @block.vector
        def _(vector):
            def rec_dve(t):
                vector.wait_ge(act_rec, t + 1)
                nc.vector.tensor_scalar_add(
                    u_sb[:], e_sb[:, (t % 2) * BL:(t % 2 + 1) * BL], 1.0)
                nc.vector.drain()
                nc.vector.reciprocal(
                    hall[:, t * BL:(t + 1) * BL], u_sb[:],
                ).then_inc(dve_h, 1)
                nc.vector.drain()
                if t % 16 == 15:
                    rb = t // 16
                    nc.vector.drain()
                    nc.vector.tensor_copy(
                        hall_r[:, rb * RBP:(rb + 1) * RBP],
                        hall[:, rb * RBP:(rb + 1) * RBP],
                    ).then_inc(dve_hr, 1)

            nbgd = [0]

            def copyB_dve(rb, c):
                g = rb * NSTG + c // CPS
                k = c % CPS
                n = nbgd[0]
                nbgd[0] += 1
                if c == 0:
                    vector.wait_ge(act_ln, rb + 1)  # logz[rb] ready
                if k == 0 and g >= 2:
                    vector.wait_ge(out_s[g % 2], OUT_INC * (g // 2))
                if n % 2 == 0:
                    vector.wait_ge(pe_pb, min(n + 2, NCH * NRB))
                nc.vector.tensor_scalar(
                    out=stg[:, (g % 2) * STG + k * CH:(g % 2) * STG + (k + 1) * CH],
                    in0=pb[n % 3][:],
                    scalar1=logz[:, rb:rb + 1], scalar2=None, op0=Sub,
                ).then_inc(dve_cb, 1)

            def reduce_rb(rb):
                vector.wait_ge(act_eA, PPB * (rb + 1))
                nc.vector.tensor_reduce(
                    rsum[:, rb:rb + 1],
                    esums[:, (rb % 2) * PPB:(rb % 2 + 1) * PPB],
                    axis=AxX, op=Add,
                ).then_inc(dve_red, 1)


            for k in range(NRB):
                vector.wait_ge(pe_xt, k + 1)
                nc.vector.tensor_copy(
                    xt[:, k * RBP:(k + 1) * RBP], pt[0:E, :],
                ).then_inc(dve_xt, 1)
            for t in range(16):
                rec_dve(t)
            # round Wo to f32r (one full-width DVE copy)
            vector.wait_ge(in_wo, 16)
            nc.vector.tensor_copy(wo_r[:], wo_f[:]).then_inc(dve_wo, 1)
            for slot in range(NRB):
                for i in range(PPB):
                    if slot >= 1:
                        copyB_dve(slot - 1, 2 * i)
                        copyB_dve(slot - 1, 2 * i + 1)
                    if slot + 1 < NRB and i % 2 == 0:
                        rec_dve(16 * (slot + 1) + i // 2)
                reduce_rb(slot)
            for i in range(PPB):
                copyB_dve(NRB - 1, 2 * i)
                copyB_dve(NRB - 1, 2 * i + 1)

    nc.finalize()
    return nc


def make_in_maps(input_batch, lookup, weight_x, weight_h, weight_o, h0):
    lookup = np.ascontiguousarray(np.asarray(lookup, dtype=np.float32))
    wx = np.asarray(weight_x, dtype=np.float32)
    wh = np.asarray(weight_h, dtype=np.float32)
    wo = np.asarray(weight_o, dtype=np.float32)
    h0T = np.ascontiguousarray(np.asarray(h0, dtype=np.float32).T)
    ident = np.eye(RBP, dtype=np.float32)
    input_batch = np.asarray(input_batch)

    # Wx/Wh stationaries replicated into the four 32-row PE strips
    wxr = np.zeros((E, RBP), np.float32)
    whr = np.zeros((H, RBP), np.float32)
    whr2 = np.zeros((H + 1, RBP), np.float32)
    woq = np.zeros((RBP, QV), np.float32)
    for q in range(4):
        wxr[:, 32 * q:32 * q + H] = wx
        whr[:, 32 * q:32 * q + H] = wh
        # r-form: h = 1 - 2r with r row16 == 0.5 exactly (strip-gap z = 0)
        whr2[0:H, 32 * q:32 * q + H] = -2.0 * wh
        whr2[H, 32 * q:32 * q + H] = 2.0 * wh.sum(axis=0)
        woq[32 * q:32 * q + H, :] = -2.0 * wo[:, q * QV:(q + 1) * QV]
        woq[32 * q + H, :] = 2.0 * wo[:, q * QV:(q + 1) * QV].sum(axis=0)

    in_maps = []
    for c in range(NCORES):
        bsl = slice(c * BL, (c + 1) * BL)
        in_maps.append({
            # idx_host[p, rb] = flat_idx[rb*128 + p] (flat is t-major: t*8+j)
            "idx": np.ascontiguousarray(
                input_batch[:, bsl].astype(np.int32).reshape(NRB, RBP).T),
            "lookup": lookup,
            "wxr": wxr,
            "whr": whr,
            "whr2": whr2,
            "woq": woq,
            "h0t": np.ascontiguousarray(h0T[:, bsl]),
            "ident": ident,
        })
    return in_maps


def kernel(input_batch, lookup, weight_x, weight_h, weight_o, h0):
    nc = build_module()
    in_maps = make_in_maps(input_batch, lookup, weight_x, weight_h, weight_o, h0)
    res = run_bass_kernel_spmd(nc, in_maps, core_ids=list(range(NCORES)))
    parts = [res.results[c]["out"].reshape(S, BL, V) for c in range(NCORES)]
    return np.concatenate(parts, axis=1)

